# revision 1
# baseline (speedup 1.0000x reference)
"""Trainium2 Bass kernel for ContextAwareMissingEmbeddingGenerator.

Data-parallel over batch: 8 cores x 512 samples. Per-row work is split into
two matmul paths:
  - scores: y = U @ x^T (184 rows, bf16) where U folds Wk against the
    constant missing-table queries; x^T is produced by the XBAR DMA-transpose
    (no PE time). Softmax runs over the free dim in [head*query, row] layout.
  - values/doc-mean: instead of folding (pred@out_proj@Wv) into a 400-row
    per-row matmul, attention weights are reduced to per-(head,row) sums
    (A9T, one-hot matmuls), expanded into block one-hot weight tiles (Abm),
    and applied as xw[d,(h,b)] = sum_r x[r,d]*Abm[r,(h,b)] using x in its
    NATURAL layout as the stationary operand. A 9th "head" carries the
    masked doc-mean. 54 tiny matmuls against (W2_h Wv_h)/S and Wp/S produce
    logits directly.
Constant parts (value bias, missing-table means, pred bias) are rank-1
matmuls against per-sample statistics.
"""

import math
from contextlib import ExitStack

import ml_dtypes
import numpy as np

import concourse.bass as bass
import concourse.bacc as bacc_mod
import concourse.mybir as mybir
import concourse.tile as tile
from concourse.bass_utils import run_bass_kernel_spmd

D, H, HD, S, L, B = 768, 8, 96, 23, 50, 4096
NCORES = 8
BC = B // NCORES              # samples per core
ROWS = BC * S                 # 11776 rows per core
NB = 16                       # samples per block
NBLK = BC // NB               # 32 blocks
N = NB * S                    # 368 rows per block (div by 16 for XBAR)
GS = [(0, 128), (128, 256), (256, 368)]   # row-chunks within a block
NH = 9                        # 8 attention heads + 1 doc-mean "head"
NEG = -30000.0

F32 = mybir.dt.float32
BF16 = mybir.dt.bfloat16
FP8 = mybir.dt.float8e4
BF = ml_dtypes.bfloat16
F8 = mybir.dt.np(mybir.dt.float8e4)
USCALE = 256.0


def _host_prep(cls_emb, missing_table, in_proj_w, in_proj_b,
               out_proj_w, out_proj_b, pred_w, pred_b, exist_mask):
    f32 = np.float32
    x = np.asarray(cls_emb, f32)
    mt = np.asarray(missing_table, f32)
    ipw = np.asarray(in_proj_w, f32)
    ipb = np.asarray(in_proj_b, f32)
    opw = np.asarray(out_proj_w, f32)
    opb = np.asarray(out_proj_b, f32)
    pw = np.asarray(pred_w, f32)
    pb = np.asarray(pred_b, f32)
    em = np.asarray(exist_mask)

    Wq, Wk, Wv = ipw[0:D], ipw[D:2 * D], ipw[2 * D:3 * D]
    bq, bk, bv = ipb[0:D], ipb[D:2 * D], ipb[2 * D:3 * D]
    scale = 1.0 / math.sqrt(HD)
    qm = mt @ Wq.T + bq
    qh = qm.reshape(S, H, HD)
    Wk3 = Wk.reshape(H, HD, D)
    Wv3 = Wv.reshape(H, HD, D)
    U = (np.einsum('hij,qhi->hqj', Wk3, qh) * scale).reshape(H * S, D)
    c0 = (np.einsum('qhi,hi->hq', qh, bk.reshape(H, HD)) * scale).reshape(H * S)
    W2 = pw @ opw
    Mcat = np.einsum('lhi,hid->hld', W2.reshape(L, H, HD), Wv3).reshape(H * L, D) / S
    vbs = ((W2 @ bv + pw @ opb) / S).astype(f32)
    wpts = ((pw @ mt.sum(0)) / S).astype(f32)

    UT = np.ascontiguousarray(U.T * USCALE).astype(F8)            # [768,184] fp8
    MT = np.ascontiguousarray(
        np.concatenate([Mcat.T, (pw / S).T], axis=1)).astype(BF)  # [768,450]
    c0col = np.ascontiguousarray(c0.reshape(H * S, 1), f32)
    k1neg = np.full((1, H * S), NEG * USCALE, f32).astype(BF)
    k1c = np.stack([vbs, wpts, pb]).astype(f32)                   # [3,50] -> flat rows
    k1c = np.ascontiguousarray(k1c.reshape(1, 3 * L))             # [1,150]

    # one-hot head reducer: rows 0..183 = (h,q) -> col h; row 184 = m -> col 8
    oh = np.zeros((H * S + 1, NH), f32)
    for i in range(H * S):
        oh[i, i // S] = 1.0
    oh[H * S, 8] = 1.0
    oh = oh.astype(BF)

    # block one-hot masks: maskm[r, h*NB + b] = 1 iff r//S == b
    maskm = np.zeros((N, NH * NB), f32)
    for r in range(N):
        b = r // S
        for h in range(NH):
            maskm[r, h * NB + b] = 1.0
    maskm = maskm.astype(BF)

    m = em.astype(f32)                                            # [B,S]
    hasany = (m.sum(1) > 0).astype(f32)
    u = (1.0 - m) * hasany[:, None]
    nupd = u.sum(1)

    xb = x.reshape(B * S, D).astype(BF)

    shards = []
    for c in range(NCORES):
        b0, b1 = c * BC, (c + 1) * BC
        xc = np.zeros((ROWS + 16, D), BF)
        xc[:ROWS] = xb[b0 * S:b1 * S]
        # scores path: pre-transposed fp8 copy, partition-major per block
        xtr = np.ascontiguousarray(
            x.reshape(B * S, D)[b0 * S:b1 * S]
            .reshape(NBLK, N, 6, 128).transpose(3, 0, 2, 1)).astype(F8)
        xtr = xtr.reshape(128, NBLK * 6 * N)
        ms = m[b0:b1].reshape(-1)                                 # [ROWS]
        # per-block packed [m-row ; (1-m)-row]
        mr2 = np.stack([ms, 1.0 - ms], axis=0).reshape(2, NBLK, N)
        mr2 = np.ascontiguousarray(mr2.transpose(1, 0, 2)).reshape(1, 2 * ROWS)
        urep = np.zeros((H * S, BC), f32)
        for h in range(H):
            urep[h * S:(h + 1) * S, :] = u[b0:b1].T
        # packed [128, 2*BC]: first half rows 0..127, second half rows
        # 128..183 on partitions 0..55 (pad rows zeroed)
        urp = np.zeros((128, 2 * BC), f32)
        urp[:, 0:BC] = urep[0:128]
        urp[0:56, BC:2 * BC] = urep[128:184]
        sv = np.stack([nupd[b0:b1], 1.0 - hasany[b0:b1],
                       np.ones(BC, f32)]).astype(f32).reshape(1, 3 * BC)
        shards.append({
            "x": xc, "xtr": xtr, "mrow": mr2.astype(BF),
            "ur": urp, "sv": np.ascontiguousarray(sv),
            "ut": UT, "mt": MT, "oh": oh, "maskm": maskm,
            "k1neg": k1neg, "c0col": c0col, "k1c": k1c,
        })
    return shards


def _build_program(ablate=5):
    nc = bacc_mod.Bacc("TRN2", target_bir_lowering=False, debug=False)
    x_d = nc.dram_tensor("x", [ROWS + 16, D], BF16, kind="ExternalInput").ap()
    xtr_d = nc.dram_tensor("xtr", [128, NBLK * 6 * N], FP8,
                           kind="ExternalInput").ap()
    mrow_d = nc.dram_tensor("mrow", [1, 2 * ROWS], BF16, kind="ExternalInput").ap()
    ur_d = nc.dram_tensor("ur", [128, 2 * BC], F32, kind="ExternalInput").ap()
    sv_d = nc.dram_tensor("sv", [1, 3 * BC], F32, kind="ExternalInput").ap()
    ut_d = nc.dram_tensor("ut", [D, H * S], FP8, kind="ExternalInput").ap()
    mt_d = nc.dram_tensor("mt", [D, NH * L], BF16, kind="ExternalInput").ap()
    oh_d = nc.dram_tensor("oh", [H * S + 1, NH], BF16, kind="ExternalInput").ap()
    mask_d = nc.dram_tensor("maskm", [N, NH * NB], BF16, kind="ExternalInput").ap()
    k1neg_d = nc.dram_tensor("k1neg", [1, H * S], BF16, kind="ExternalInput").ap()
    c0_d = nc.dram_tensor("c0col", [H * S, 1], F32, kind="ExternalInput").ap()
    k1c_d = nc.dram_tensor("k1c", [1, 3 * L], F32, kind="ExternalInput").ap()
    out_d = nc.dram_tensor("logitsT", [L, BC], F32, kind="ExternalOutput").ap()

    EXP = mybir.ActivationFunctionType.Exp

    def mm(out, lhsT, rhs, start, stop):
        nc.tensor.matmul(out, lhsT, rhs, start=start, stop=stop)

    with tile.TileContext(nc) as tc, ExitStack() as ctx:
        cpool = ctx.enter_context(tc.tile_pool(name="consts", bufs=1))
        natp = ctx.enter_context(tc.tile_pool(name="xnat", bufs=10))
        xtp = ctx.enter_context(tc.tile_pool(name="xts", bufs=11))
        ewp = ctx.enter_context(tc.tile_pool(name="ew", bufs=6))
        wwp = ctx.enter_context(tc.tile_pool(name="ww", bufs=20))
        smp = ctx.enter_context(tc.tile_pool(name="small", bufs=6))
        abp = ctx.enter_context(tc.tile_pool(name="abm", bufs=6))
        xsp = ctx.enter_context(tc.tile_pool(name="xwsb", bufs=6))
        outp = ctx.enter_context(tc.tile_pool(name="outp", bufs=1))
        yp = ctx.enter_context(tc.tile_pool(name="ypsum", bufs=2, space="PSUM"))
        y1p = ctx.enter_context(tc.tile_pool(name="y1psum", bufs=2, space="PSUM"))
        xwp = ctx.enter_context(tc.tile_pool(name="xwpsum", bufs=2, space="PSUM"))
        a9p = ctx.enter_context(tc.tile_pool(name="a9psum", bufs=1, space="PSUM"))
        ctp = ctx.enter_context(tc.tile_pool(name="ctpsum", bufs=1, space="PSUM"))

        def cload(name, shape, src, cdt=F32):
            t = cpool.tile(shape, cdt, tag=name, name=name)
            nc.sync.dma_start(t[:], src)
            return t

        ut_sb = [cload(f"ut{dc}", [128, H * S], ut_d[dc * 128:(dc + 1) * 128, :], FP8)
                 for dc in range(6)]
        mt_sb = [cload(f"mt{dc}", [128, NH * L], mt_d[dc * 128:(dc + 1) * 128, :], BF16)
                 for dc in range(6)]
        oh0 = cload("oh0", [128, NH], oh_d[0:128, :], BF16)
        oh1 = cload("oh1", [56, NH], oh_d[128:184, :], BF16)
        ohm = cload("ohm", [1, NH], oh_d[184:185, :], BF16)
        km = [cload(f"km{g}", [g1 - g0, NH * NB], mask_d[g0:g1, :], BF16)
              for g, (g0, g1) in enumerate(GS)]
        k1neg_sb = cload("k1neg", [1, H * S], k1neg_d[:, :], BF16)
        c00 = cload("c00", [128, 1], c0_d[0:128, :])
        c01 = cload("c01", [56, 1], c0_d[128:184, :])
        urp = cload("urp", [128, 2 * BC], ur_d[:, :])
        sv_sb = cload("sv", [1, 3 * BC], sv_d[:, :])
        mrow_sb = cload("mrow", [1, 2 * ROWS], mrow_d[:, :], BF16)
        k1c_sb = cload("k1c", [1, 3 * L], k1c_d[:, :])
        outT = outp.tile([L, BC], F32, tag="outT", name="outT")

        def mark(blk, phase):
            # build-time phase markers for offline timeline attribution;
            # disabled in production builds (full instruction scans are slow)
            if PHASES_ENABLED:
                PHASES.append((blk, phase, list(nc.all_instructions())[-1].name))

        # per-block tile handles, keyed by block index
        tiles = {}

        def emit_xts(b):
            # scores operand: host-pre-transposed fp8 copy, plain load
            xts = xtp.tile([128, 6, N], FP8, tag="xts", name=f"xts{b}")
            nc.sync.dma_start(xts[:], xtr_d[:, b * 6 * N:(b + 1) * 6 * N])
            tiles.setdefault(b, {})["xts"] = xts

        def emit_xnat(b):
            xnat = natp.tile([128, 3, D], BF16, tag="xnat", name=f"xnat{b}")
            nc.scalar.dma_start(
                xnat[:], x_d[b * N:b * N + 384, :].rearrange("(g p) d -> p g d", p=128))
            tiles.setdefault(b, {})["xnat"] = xnat
            mark(b, 'dma')

        def emit_scores(b):
            r0 = b * N
            xts = tiles[b]["xts"]
            mrn = mrow_sb[:, 2 * r0 + N:2 * r0 + 2 * N]
            y0 = yp.tile([128, N], F32, tag="y0", name=f"y0_{b}")
            y1 = y1p.tile([56, N], F32, tag="y1", name=f"y1_{b}")
            with tc.high_priority():
                for dc in range(6):
                    mm(y0[:], ut_sb[dc][:, 0:128], xts[:, dc, :],
                       start=(dc == 0), stop=False)
                mm(y0[:], k1neg_sb[:, 0:128], mrn, start=False, stop=True)
                for dc in range(6):
                    mm(y1[:], ut_sb[dc][:, 128:184], xts[:, dc, :],
                       start=(dc == 0), stop=False)
                mm(y1[:], k1neg_sb[:, 128:184], mrn, start=False, stop=True)
            tiles[b].update(y0=y0, y1=y1)
            mark(b, 'y')

        def emit_softmax(b):
            b0 = b * NB
            y0, y1 = tiles[b]["y0"], tiles[b]["y1"]

            # packed halves: [:, 0:N] = rows 0..127, [0:56, N:2N] = rows
            # 128..183. Pad rows (56:128 of the second half) hold garbage
            # that is masked by urp's zero pad and never read downstream.
            ew = ewp.tile([128, 2 * N], BF16, tag="ew", name=f"ew{b}")
            nc.scalar.activation(ew[:, 0:N], y0[:], EXP, bias=c00[:],
                                 scale=1.0 / USCALE)
            nc.scalar.activation(ew[0:56, N:2 * N], y1[:], EXP, bias=c01[:],
                                 scale=1.0 / USCALE)
            den = smp.tile([128, 2 * NB], F32, tag="den", name=f"den{b}")
            nc.vector.tensor_reduce(
                den[:].rearrange("p (t b) -> p t b", t=2),
                ew[:].rearrange("p (t b k) -> p t b k", t=2, k=S),
                axis=mybir.AxisListType.X, op=mybir.AluOpType.add)
            nc.vector.tensor_scalar_add(den[:], den[:], 1e-30)
            up = smp.tile([128, 2 * NB], F32, tag="up", name=f"up{b}")
            nc.vector.reciprocal(up[:], den[:])
            nc.gpsimd.tensor_mul(
                up[:].rearrange("p (t b) -> p t b", t=2),
                up[:].rearrange("p (t b) -> p t b", t=2),
                urp[:].rearrange("p (t c) -> p t c", t=2)[:, :, b0:b0 + NB])
            w = wwp.tile([128, 2 * N], BF16, tag="w", name=f"w{b}")
            bc = (up[:].rearrange("p (t b o) -> p t b o", t=2, o=1)
                  .broadcast_to([128, 2, NB, S]))
            # halves run in parallel on DVE and Pool
            nc.vector.tensor_mul(
                w[:, 0:N].rearrange("p (b k) -> p b k", k=S),
                ew[:, 0:N].rearrange("p (b k) -> p b k", k=S), bc[:, 0, :, :])
            nc.gpsimd.tensor_mul(
                w[0:56, N:2 * N].rearrange("p (b k) -> p b k", k=S),
                ew[0:56, N:2 * N].rearrange("p (b k) -> p b k", k=S),
                bc[0:56, 1, :, :])
            tiles[b].update(w=w)
            mark(b, 'w')

        def emit_a9ab(b):
            w = tiles[b]["w"]
            mrp = mrow_sb[:, 2 * b * N:2 * b * N + N]
            # per-(row, head) weight sums, 9th col = m (doc-mean).
            # All matmuls first, then all ab reads — interleaving would
            # create tile-granular WAR stalls between PE and DVE.
            a9 = a9p.tile([128, 3 * NH], F32, tag="a9", name=f"a9_{b}")
            for g, (g0, g1) in enumerate(GS):
                sl = a9[0:g1 - g0, g * NH:(g + 1) * NH]
                mm(sl, w[:, g0:g1], oh0[:], start=True, stop=False)
                mm(sl, w[0:56, N + g0:N + g1], oh1[:], start=False, stop=False)
                mm(sl, mrp[:, g0:g1], ohm[:], start=False, stop=True)
            # drain a9 out of PSUM immediately (collapses the a9-reuse
            # cycle to this one copy) so the expansion multiplies can run
            # on the idle Pool engine from SBUF
            a9s = smp.tile([128, 3 * NH], BF16, tag="a9s", name=f"a9s_{b}")
            nc.vector.tensor_copy(a9s[:], a9[:])
            ab = []
            for g, (g0, g1) in enumerate(GS):
                pg = g1 - g0
                abg = abp.tile([pg, NH, NB], BF16, tag=f"ab{g}", name=f"ab{b}_{g}")
                nc.vector.tensor_mul(
                    abg[:],
                    a9s[0:pg, g * NH:(g + 1) * NH]
                    .rearrange("p (j o) -> p j o", o=1).broadcast_to([pg, NH, NB]),
                    km[g][:].rearrange("p (j b) -> p j b", b=NB))
                ab.append(abg)
            tiles[b]["ab"] = ab
            mark(b, 'ab')

        def emit_xw(b):
            ab = tiles[b]["ab"]
            xnat = tiles[b]["xnat"]
            # xw[d, (h,b)] = sum_r x[r,d] * Abm[r,(h,b)] — two half-passes
            # (dc 0-2, dc 3-5) over one double-buffered PSUM tag. Sample
            # spans per row-chunk: g0 rows cover b 0..5, g1 b 5..11, g2 b
            # 11..15. g0 writes full width (start resets the bank region);
            # g1/g2 accumulate only their local columns.
            SPAN = [(0, NB), (5, 12), (11, NB)]
            xss = []
            for half in range(2):
                xwh = xwp.tile([128, 3, NH, NB], F32, tag="xwH",
                               name=f"xw{b}_{half}")
                for dci in range(3):
                    for g, (g0, g1) in enumerate(GS):
                        bo0, bo1 = SPAN[g]
                        o = xwh[:, dci, :, bo0:bo1]
                        dc = half * 3 + dci
                        mm(o, xnat[0:g1 - g0, g, dc * 128:(dc + 1) * 128],
                           ab[g][:, :, bo0:bo1], start=(g == 0), stop=(g == 2))
                xsh = xsp.tile([128, 3, NH, NB], BF16, tag=f"xs{half}",
                               name=f"xs{b}_{half}")
                if half == 0:
                    nc.scalar.copy(xsh[:], xwh[:])
                else:
                    nc.vector.tensor_copy(xsh[:], xwh[:])
                xss.append(xsh)
            tiles[b]["xs"] = xss
            mark(b, 'xs')

        def emit_mh(b):
            b0 = b * NB
            xsA, xsB = tiles[b]["xs"]
            # logits: 54 head matmuls + 3 rank-1 constant terms
            ct = ctp.tile([L, NB], F32, tag="ct", name=f"ct{b}")
            first = True
            for dc in range(6):
                xsb = xsA if dc < 3 else xsB
                for h in range(NH):
                    mm(ct[:], mt_sb[dc][:, h * L:(h + 1) * L], xsb[:, dc % 3, h, :],
                       start=first, stop=False)
                    first = False
            for j in range(3):
                mm(ct[:], k1c_sb[:, j * L:(j + 1) * L],
                   sv_sb[:, j * BC + b0:j * BC + b0 + NB],
                   start=False, stop=(j == 2))
            nc.vector.tensor_copy(outT[:, b0:b0 + NB], ct[:])
            del tiles[b]
            mark(b, 'ct')

        # software-pipelined emission: the scheduler's priority follows
        # emission order, so loads run 3 blocks ahead, scores 2, softmax 1
        # ahead of the current block's value-path consumption
        for b in range(min(5, NBLK)):
            emit_xts(b)
        for b in range(min(3, NBLK)):
            emit_xnat(b)
        emit_scores(0)
        emit_scores(1)
        if ablate >= 2:
            emit_softmax(0)
        for blk in range(NBLK):
            if blk + 5 < NBLK:
                emit_xts(blk + 5)
            if blk + 3 < NBLK:
                emit_xnat(blk + 3)
            if blk + 2 < NBLK:
                emit_scores(blk + 2)
            if blk + 1 < NBLK:
                emit_softmax(blk + 1)
            if blk < NBLK:
                emit_a9ab(blk)
                emit_xw(blk)
                emit_mh(blk)

        nc.sync.dma_start(out_d[:, :], outT[:])
    nc.compile()
    return nc


PHASES = []
PHASES_ENABLED = False


_CACHED = {}


def _get_program():
    if "nc" not in _CACHED:
        _CACHED["nc"] = _build_program()
    return _CACHED["nc"]


def _run(inputs, trace=False):
    shards = _host_prep(**inputs)
    nc = _get_program()
    res = run_bass_kernel_spmd(nc, shards, list(range(NCORES)), trace=trace)
    outs = [res.results[i]["logitsT"] for i in range(NCORES)]
    full = np.concatenate(outs, axis=1).T.astype(np.float32)
    return full, res


def kernel(**inputs):
    out, _ = _run(inputs, trace=False)
    return out


def run_traced(inputs):
    return _run(inputs, trace=True)



# revision 2
# speedup vs baseline: 1.2439x; 1.2439x over previous
"""Trainium2 Bass kernel for ContextAwareMissingEmbeddingGenerator (v2).

Data-parallel over batch: 8 cores x 512 samples. Dispatch time is dominated
by host->device transfer over the axon tunnel, so the design minimizes
shipped bytes:

  - Only EXISTING sections' cls_emb rows are shipped (missing rows are
    replaced by the constant missing table and provably never read). Rows
    are bin-packed on host into NBLK blocks of NKEY=256 key slots spanning
    <= MS=26 whole samples per block (best-fit decreasing on section count).
  - x ships once, as int8 (delta = 4.5/127). On device it is cast to bf16
    (value path) and PE-transposed + downcast to fp8 (scores path).
  - Scores are computed in [key, score] orientation: softmax denominators
    and the per-key expansion of 1/den both become small matmuls against a
    per-block key->sample one-hot built on device via is_equal(iota, ids).
  - The doc-mean path is linear in x, so the int8 quantization error there
    is cancelled exactly by a host-computed per-sample correction, folded
    together with all constant terms (value bias, missing-table means,
    predictor bias) into one [L, col] array added at output time.

Math is the same folding as v1: scores y[key,(h,q)] = x_key . U[(h,q)] with
U = scale * Wk^T q(missing_table); per-key per-head attention mass a9 feeds
block one-hot expansion ab; xw[d,(h,col)] = sum_key x[key,d] ab[key,(h,col)]
contracts against (W2_h Wv_h)/S and pred/S to yield logits directly. A 9th
"head" carries the masked doc-mean. Output columns are block-packed; the
host scatters them back to sample order.
"""

import math
import os
import tempfile
from contextlib import ExitStack

import ml_dtypes
import numpy as np

# Re-jitting the dispatch wrapper every call costs ~0.5s in XLA/neuronx
# recompilation; the persistent cache turns that into a fast disk hit.
try:
    import jax
    _cdir = os.path.join(tempfile.gettempdir(), "jax_comp_cache_kernel")
    os.makedirs(_cdir, exist_ok=True)
    jax.config.update("jax_compilation_cache_dir", _cdir)
    jax.config.update("jax_persistent_cache_min_compile_time_secs", 0.0)
    jax.config.update("jax_persistent_cache_min_entry_size_bytes", 0)
except Exception:
    pass

import concourse.bass as bass
import concourse.bacc as bacc_mod
import concourse.mybir as mybir
import concourse.tile as tile
from concourse.bass_utils import run_bass_kernel_spmd

D, H, HD, S, L, B = 768, 8, 96, 23, 50, 4096
NCORES = 8
BC = B // NCORES              # samples per core
NBLK = 24                     # packed blocks per core
NKEY = 256                    # key-row slots per block (2 tiles of 128)
MS = 26                       # max samples per block
NSC = NBLK * MS               # output column slots per core
NH = 9                        # 8 attention heads + 1 doc-mean "head"
NEG = -30000.0
USCALE = 256.0
DELTA = 4.5 / 127.0           # int8 quantization step for x

F32 = mybir.dt.float32
BF16 = mybir.dt.bfloat16
FP8 = mybir.dt.float8e4
I8 = mybir.dt.int8
BF = ml_dtypes.bfloat16
F8 = ml_dtypes.float8_e4m3


def _pack_core(cm):
    """Best-fit-decreasing bin packing of 512 samples into NBLK blocks.

    cm: [BC, S] float 0/1 existing mask. Returns (bins, cnt) where bins is a
    list of NBLK lists of sample indices (placement order = local column).
    """
    cnt = cm.sum(1).astype(np.int64)
    order = np.argsort(-cnt, kind="stable")
    keys_used = np.zeros(NBLK, np.int64)
    samp_used = np.zeros(NBLK, np.int64)
    bins = [[] for _ in range(NBLK)]
    for s in order:
        ok = (keys_used + cnt[s] <= NKEY) & (samp_used < MS)
        if not ok.any():
            raise RuntimeError("bin packing infeasible; raise NBLK")
        cand = np.where(ok)[0]
        j = cand[np.argmax(keys_used[cand])]
        bins[j].append(int(s))
        keys_used[j] += cnt[s]
        samp_used[j] += 1
    return bins, cnt


def _host_prep(cls_emb, missing_table, in_proj_w, in_proj_b,
               out_proj_w, out_proj_b, pred_w, pred_b, exist_mask):
    f32 = np.float32
    x = np.asarray(cls_emb, f32)
    mt = np.asarray(missing_table, f32)
    ipw = np.asarray(in_proj_w, f32)
    ipb = np.asarray(in_proj_b, f32)
    opw = np.asarray(out_proj_w, f32)
    opb = np.asarray(out_proj_b, f32)
    pw = np.asarray(pred_w, f32)
    pb = np.asarray(pred_b, f32)
    em = np.asarray(exist_mask)

    Wq, Wk, Wv = ipw[0:D], ipw[D:2 * D], ipw[2 * D:3 * D]
    bq, bk, bv = ipb[0:D], ipb[D:2 * D], ipb[2 * D:3 * D]
    scale = 1.0 / math.sqrt(HD)
    qm = mt @ Wq.T + bq
    qh = qm.reshape(S, H, HD)
    Wk3 = Wk.reshape(H, HD, D)
    Wv3 = Wv.reshape(H, HD, D)
    U = (np.einsum('hij,qhi->hqj', Wk3, qh) * scale).reshape(H * S, D)
    c0 = (np.einsum('qhi,hi->hq', qh, bk.reshape(H, HD)) * scale).reshape(H * S)
    W2 = pw @ opw
    Mcat = np.einsum('lhi,hid->hld', W2.reshape(L, H, HD), Wv3).reshape(H * L, D) / S
    vbs = ((W2 @ bv + pw @ opb) / S).astype(f32)          # [L] per updated query
    wpts = ((pw @ mt.sum(0)) / S).astype(f32)             # [L] full-table mean

    UT = np.ascontiguousarray(U.T * USCALE).astype(F8)    # [768, 184]
    MT = np.ascontiguousarray(
        np.concatenate([Mcat.T, (pw / S).T], axis=1)).astype(BF)  # [768, 450]
    cn2 = np.stack([c0 * USCALE,
                    np.full(H * S, NEG * USCALE, f32)]).astype(BF)  # [2, 184]
    idbf = np.eye(128, dtype=BF)
    iota_mat = np.ascontiguousarray(
        np.broadcast_to(np.arange(MS, dtype=f32), (128, MS)))

    m = em.astype(f32)                                    # [B, S]
    hasany = (m.sum(1) > 0).astype(f32)
    u = (1.0 - m) * hasany[:, None]
    nupd = u.sum(1)

    # int8 quantized x and the exact doc-mean correction (linear in x)
    xq = np.clip(np.rint(x * (1.0 / DELTA)), -127, 127).astype(np.int8)
    errsum = (np.einsum('bsd,bs->bd', x, m, optimize=True)
              - DELTA * np.einsum('bsd,bs->bd', xq.astype(f32), m, optimize=True))
    corr_all = (errsum @ pw.T) / S \
        + nupd[:, None] * vbs + (1.0 - hasany)[:, None] * wpts + pb  # [B, L]

    shards = []
    colmaps = []
    for c in range(NCORES):
        b0 = c * BC
        cm = m[b0:b0 + BC]
        bins, cnt = _pack_core(cm)
        xpack = np.zeros((NBLK * NKEY, D), np.int8)
        sampcol = np.full((128, NBLK * 2), -1.0, f32)
        realcol = np.zeros((128, NBLK * 2), f32)
        rc2 = np.zeros((2, NBLK * NKEY), f32)
        urpk = np.zeros((MS, NBLK * S), f32)
        corrpk = np.zeros((L, NSC), f32)
        colmap = np.zeros(BC, np.int64)
        for bI, samples in enumerate(bins):
            r = 0
            for j, s in enumerate(samples):
                g = b0 + s
                secs = np.nonzero(cm[s])[0]
                n = len(secs)
                rr = bI * NKEY + r
                xpack[rr:rr + n] = xq[g, secs]
                pidx = np.arange(r, r + n)
                sampcol[pidx % 128, bI * 2 + pidx // 128] = j
                realcol[pidx % 128, bI * 2 + pidx // 128] = 1.0
                rc2[0, rr:rr + n] = 1.0
                urpk[j, bI * S:(bI + 1) * S] = u[g]
                corrpk[:, bI * MS + j] = corr_all[g]
                colmap[s] = bI * MS + j
                r += n
        rc2[1] = 1.0 - rc2[0]
        xqh = np.ascontiguousarray(
            xpack.reshape(NBLK, 2, 128, D).transpose(2, 0, 1, 3)
        ).reshape(128, NBLK * 2 * D)
        pieces = {
            "xq": xqh, "ut": UT, "mt": MT,
            "corr": corrpk.astype(BF), "urpk": urpk.astype(BF),
            "rc2": rc2.astype(BF), "cn2": cn2,
            "sampcol": sampcol.astype(BF), "realcol": realcol.astype(BF),
            "iota": iota_mat.astype(BF), "idbf": idbf,
        }
        buf = np.empty(_AUX_BYTES, np.uint8)
        for name, _, _, ob, nb in _AUX_LAYOUT:
            buf[ob:ob + nb] = pieces[name].reshape(-1).view(np.uint8)
        shards.append({
            "aux": buf.view(BF).reshape(1, -1),
            # extra key, ignored by run_bass_kernel_spmd (only declared
            # input names are read); used by _run to unscatter columns
            "colmap": colmap,
        })
    return shards


def _aux_layout():
    """(name, np_dtype_bytes, shape, byte_offset, byte_count) for each piece
    packed into the single bf16 "aux" input tensor."""
    items = [
        ("xq", 1, (128, NBLK * 2 * D)),
        ("ut", 1, (D, H * S)),
        ("mt", 2, (D, NH * L)),
        ("corr", 2, (L, NSC)),
        ("urpk", 2, (MS, NBLK * S)),
        ("rc2", 2, (2, NBLK * NKEY)),
        ("cn2", 2, (2, H * S)),
        ("sampcol", 2, (128, NBLK * 2)),
        ("realcol", 2, (128, NBLK * 2)),
        ("iota", 2, (128, MS)),
        ("idbf", 2, (128, 128)),
    ]
    out = []
    off = 0
    for name, isz, shape in items:
        nb = isz * int(np.prod(shape))
        assert nb % 2 == 0
        out.append((name, isz, shape, off, nb))
        off += nb
    return out, off


_AUX_LAYOUT, _AUX_BYTES = _aux_layout()


def _build_program():
    nc = bacc_mod.Bacc("TRN2", target_bir_lowering=False, debug=False)
    aux_d = nc.dram_tensor("aux", [1, _AUX_BYTES // 2], BF16,
                           kind="ExternalInput").ap()

    def aux_view(name):
        (_, isz, shape, ob, nb) = next(e for e in _AUX_LAYOUT if e[0] == name)
        v = aux_d[:, ob // 2:(ob + nb) // 2]
        if isz == 1:
            v = v.bitcast(I8 if name == "xq" else FP8)
        return v.rearrange("o (p f) -> (o p) f", p=shape[0])

    xq_d = aux_view("xq")
    sampcol_d = aux_view("sampcol")
    realcol_d = aux_view("realcol")
    rc2_d = aux_view("rc2")
    cn2_d = aux_view("cn2")
    urpk_d = aux_view("urpk")
    corr_d = aux_view("corr")
    iota_d = aux_view("iota")
    idbf_d = aux_view("idbf")
    ut_d = aux_view("ut")
    mt_d = aux_view("mt")
    out_d = nc.dram_tensor("logitsT", [L, NSC], F32, kind="ExternalOutput").ap()

    EXP = mybir.ActivationFunctionType.Exp
    COPY = mybir.ActivationFunctionType.Copy
    HS = H * S

    def mm(out, lhsT, rhs, start, stop):
        nc.tensor.matmul(out, lhsT, rhs, start=start, stop=stop)

    with tile.TileContext(nc) as tc, ExitStack() as ctx:
        cpool = ctx.enter_context(tc.tile_pool(name="consts", bufs=1))
        xip = ctx.enter_context(tc.tile_pool(name="xi", bufs=3))
        xnp = ctx.enter_context(tc.tile_pool(name="xn", bufs=2))
        xtp = ctx.enter_context(tc.tile_pool(name="xt", bufs=2))
        ohp = ctx.enter_context(tc.tile_pool(name="oh", bufs=2))
        ohtp = ctx.enter_context(tc.tile_pool(name="oht", bufs=2))
        ewp = ctx.enter_context(tc.tile_pool(name="ew", bufs=2))
        wvp = ctx.enter_context(tc.tile_pool(name="wv", bufs=2))
        smp = ctx.enter_context(tc.tile_pool(name="small", bufs=4))
        abp = ctx.enter_context(tc.tile_pool(name="ab", bufs=2))
        xsp = ctx.enter_context(tc.tile_pool(name="xs", bufs=2))
        outp = ctx.enter_context(tc.tile_pool(name="out", bufs=1))
        tpp = ctx.enter_context(tc.tile_pool(name="tpps", bufs=2, space="PSUM"))
        top = ctx.enter_context(tc.tile_pool(name="topsum", bufs=1, space="PSUM"))
        yp = ctx.enter_context(tc.tile_pool(name="ypsum", bufs=2, space="PSUM"))
        spp = ctx.enter_context(tc.tile_pool(name="spsum", bufs=2, space="PSUM"))
        xwp = ctx.enter_context(tc.tile_pool(name="xwpsum", bufs=1, space="PSUM"))

        def cload(name, shape, src, cdt=F32):
            t = cpool.tile(shape, cdt, tag=name, name=name)
            nc.sync.dma_start(t[:], src)
            return t

        ut_sb = [cload(f"ut{dc}", [128, HS], ut_d[dc * 128:(dc + 1) * 128, :], FP8)
                 for dc in range(6)]
        mt_sb = [cload(f"mt{dc}", [128, NH * L], mt_d[dc * 128:(dc + 1) * 128, :],
                       BF16) for dc in range(6)]
        rc2_sb = cload("rc2", [2, NBLK * NKEY], rc2_d[:, :], BF16)
        cn2_sb = cload("cn2", [2, HS], cn2_d[:, :], BF16)
        urpk_sb = cload("urpk", [MS, NBLK * S], urpk_d[:, :], BF16)
        corr_sb = cload("corr", [L, NSC], corr_d[:, :], BF16)
        iota_sb = cload("iota", [128, MS], iota_d[:, :], BF16)
        idbf_sb = cload("idbf", [128, 128], idbf_d[:, :], BF16)
        sampcol_sb = cload("sampcol", [128, NBLK * 2], sampcol_d[:, :], BF16)
        realcol_sb = cload("realcol", [128, NBLK * 2], realcol_d[:, :], BF16)
        outT = outp.tile([L, NSC], F32, tag="outT", name="outT")

        tiles = {}

        def emit_load(b):
            xi = xip.tile([128, 2, D], I8, tag="xi", name=f"xi{b}")
            nc.sync.dma_start(xi[:], xq_d[:, b * 2 * D:(b + 1) * 2 * D]
                              .rearrange("p (t d) -> p t d", t=2))
            tiles.setdefault(b, {})["xi"] = xi

        def emit_cast(b):
            xi = tiles[b]["xi"]
            xnb = xnp.tile([128, 2, D], BF16, tag="xnb", name=f"xnb{b}")
            nc.scalar.activation(xnb[:], xi[:], COPY, scale=DELTA)
            tiles[b]["xnb"] = xnb

        def emit_xts(b):
            xnb = tiles[b]["xnb"]
            xts = xtp.tile([128, 6, NKEY], FP8, tag="xts", name=f"xts{b}")
            for t in range(2):
                for dc in range(6):
                    tp = tpp.tile([128, 128], BF16, tag="tp", name=f"tp{b}_{t}_{dc}")
                    nc.tensor.transpose(tp[:], xnb[:, t, dc * 128:(dc + 1) * 128],
                                        idbf_sb[:])
                    if dc % 2 == 0:
                        nc.scalar.copy(xts[:, dc, t * 128:(t + 1) * 128], tp[:])
                    else:
                        nc.vector.tensor_copy(xts[:, dc, t * 128:(t + 1) * 128],
                                              tp[:])
            tiles[b]["xts"] = xts

        def emit_oh(b):
            oh = ohp.tile([128, 2, MS], BF16, tag="oh", name=f"oh{b}")
            ohT = ohtp.tile([MS, 2, 128], BF16, tag="ohT", name=f"ohT{b}")
            for t in range(2):
                nc.vector.tensor_tensor(
                    oh[:, t, :], iota_sb[:],
                    sampcol_sb[:, b * 2 + t:b * 2 + t + 1].broadcast_to([128, MS]),
                    op=mybir.AluOpType.is_equal)
                tpo = top.tile([MS, 128], BF16, tag="tpo", name=f"tpo{b}_{t}")
                nc.tensor.transpose(tpo[:], oh[:, t, :], idbf_sb[:])
                nc.vector.tensor_copy(ohT[:, t, :], tpo[:])
            tiles[b]["oh"] = oh
            tiles[b]["ohT"] = ohT

        def emit_scores(b):
            xts = tiles[b]["xts"]
            ewT = ewp.tile([128, 2, HS], BF16, tag="ewT", name=f"ewT{b}")
            for t in range(2):
                y = yp.tile([128, HS], F32, tag="y", name=f"y{b}_{t}")
                with tc.high_priority():
                    for dc in range(6):
                        mm(y[:], xts[:, dc, t * 128:(t + 1) * 128], ut_sb[dc][:],
                           start=(dc == 0), stop=False)
                    r0 = b * NKEY + t * 128
                    mm(y[:], rc2_sb[:, r0:r0 + 128], cn2_sb[:],
                       start=False, stop=True)
                nc.scalar.activation(ewT[:, t, :], y[:], EXP, scale=1.0 / USCALE)
            tiles[b]["ewT"] = ewT

        def emit_den(b):
            oh, ewT = tiles[b]["oh"], tiles[b]["ewT"]
            dpsf = spp.tile([128, HS], F32, tag="sp", name=f"dps{b}")
            dps = dpsf[0:MS, :]
            mm(dps, oh[:, 0, :], ewT[:, 0, :], start=True, stop=False)
            mm(dps, oh[:, 1, :], ewT[:, 1, :], start=False, stop=True)
            up = smp.tile([MS, HS], F32, tag="up", name=f"up{b}")
            nc.vector.tensor_scalar_add(up[:], dps, 1e-30)
            nc.vector.reciprocal(up[:], up[:])
            upu = smp.tile([MS, HS], BF16, tag="upu", name=f"upu{b}")
            nc.gpsimd.tensor_mul(
                upu[:].rearrange("p (h k) -> p h k", k=S),
                up[:].rearrange("p (h k) -> p h k", k=S),
                urpk_sb[:, b * S:(b + 1) * S]
                .rearrange("p (o k) -> p o k", o=1).broadcast_to([MS, H, S]))
            tiles[b]["upu"] = upu

        def emit_w(b):
            ohT, ewT, upu = tiles[b]["ohT"], tiles[b]["ewT"], tiles[b]["upu"]
            wv = wvp.tile([128, 2, HS], BF16, tag="wv", name=f"wv{b}")
            for t in range(2):
                ue = spp.tile([128, HS], F32, tag="sp", name=f"ue{b}_{t}")
                mm(ue[:], ohT[:, t, :], upu[:], start=True, stop=True)
                nc.vector.tensor_mul(wv[:, t, :], ewT[:, t, :], ue[:])
            tiles[b]["wv"] = wv

        def emit_a9ab(b):
            wv, oh = tiles[b]["wv"], tiles[b]["oh"]
            a9 = smp.tile([128, 2, NH], F32, tag="a9", name=f"a9_{b}")
            nc.vector.tensor_reduce(
                a9[:, :, 0:H].rearrange("p t (h o) -> p t h o", o=1),
                wv[:].rearrange("p t (h k) -> p t h k", k=S),
                axis=mybir.AxisListType.X, op=mybir.AluOpType.add)
            nc.gpsimd.tensor_copy(
                a9[:, :, H:NH],
                realcol_sb[:, b * 2:b * 2 + 2].rearrange("p (t o) -> p t o", o=1))
            ab = abp.tile([128, 2, NH, MS], BF16, tag="ab", name=f"ab{b}")
            nc.gpsimd.tensor_copy(
                ab[:],
                oh[:].rearrange("p t (o m) -> p t o m", o=1)
                .broadcast_to([128, 2, NH, MS]))
            nc.gpsimd.tensor_mul(
                ab[:], ab[:],
                a9[:].rearrange("p t (h o) -> p t h o", o=1)
                .broadcast_to([128, 2, NH, MS]))
            tiles[b]["ab"] = ab

        def emit_xw(b):
            xnb, ab = tiles[b]["xnb"], tiles[b]["ab"]
            xs = xsp.tile([128, 6, NH * MS], BF16, tag="xs", name=f"xs{b}")
            for dc in range(6):
                xw = xwp.tile([128, NH * MS], F32, tag="xw", name=f"xw{b}_{dc}")
                mm(xw[:], xnb[:, 0, dc * 128:(dc + 1) * 128], ab[:, 0],
                   start=True, stop=False)
                mm(xw[:], xnb[:, 1, dc * 128:(dc + 1) * 128], ab[:, 1],
                   start=False, stop=True)
                if dc % 2 == 0:
                    nc.scalar.copy(xs[:, dc, :], xw[:])
                else:
                    nc.vector.tensor_copy(xs[:, dc, :], xw[:])
            tiles[b]["xs"] = xs

        def emit_mh(b):
            xs = tiles[b]["xs"]
            ctf = spp.tile([128, HS], F32, tag="sp", name=f"ct{b}")
            ct = ctf[0:L, 0:MS]
            first = True
            for dc in range(6):
                for h in range(NH):
                    mm(ct, mt_sb[dc][:, h * L:(h + 1) * L],
                       xs[:, dc, h * MS:(h + 1) * MS],
                       start=first, stop=(dc == 5 and h == NH - 1))
                    first = False
            nc.vector.tensor_tensor(
                outT[:, b * MS:(b + 1) * MS], ct,
                corr_sb[:, b * MS:(b + 1) * MS], op=mybir.AluOpType.add)
            del tiles[b]

        for b in range(min(2, NBLK)):
            emit_load(b)
        emit_cast(0)
        for b in range(NBLK):
            if b + 2 < NBLK:
                emit_load(b + 2)
            if b + 1 < NBLK:
                emit_cast(b + 1)
            emit_xts(b)
            emit_oh(b)
            emit_scores(b)
            emit_den(b)
            emit_w(b)
            emit_a9ab(b)
            emit_xw(b)
            emit_mh(b)

        nc.sync.dma_start(out_d[:, :], outT[:])
    nc.compile()
    return nc


_CACHED = {}


def _get_program():
    if "nc" not in _CACHED:
        _CACHED["nc"] = _build_program()
    return _CACHED["nc"]


def _run(inputs, trace=False):
    shards = _host_prep(**inputs)
    nc = _get_program()
    res = run_bass_kernel_spmd(nc, shards, list(range(NCORES)), trace=trace)
    full = np.empty((B, L), np.float32)
    for c in range(NCORES):
        oT = res.results[c]["logitsT"]          # [L, NSC]
        full[c * BC:(c + 1) * BC] = oT[:, shards[c]["colmap"]].T
    return full, res


def kernel(**inputs):
    out, _ = _run(inputs, trace=False)
    return out


def run_traced(inputs):
    return _run(inputs, trace=True)


# revision 3
# speedup vs baseline: 1.4246x; 1.1453x over previous
"""Trainium2 Bass kernel for ContextAwareMissingEmbeddingGenerator (v2).

Data-parallel over batch: 8 cores x 512 samples. Dispatch time is dominated
by host->device transfer over the axon tunnel, so the design minimizes
shipped bytes:

  - Only EXISTING sections' cls_emb rows are shipped (missing rows are
    replaced by the constant missing table and provably never read). Rows
    are bin-packed on host into NBLK blocks of NKEY=256 key slots spanning
    <= MS=26 whole samples per block (best-fit decreasing on section count).
  - x ships once, as int8 (delta = 4.5/127). On device it is cast to bf16
    (value path) and PE-transposed + downcast to fp8 (scores path).
  - Scores are computed in [key, score] orientation: softmax denominators
    and the per-key expansion of 1/den both become small matmuls against a
    per-block key->sample one-hot built on device via is_equal(iota, ids).
  - The doc-mean path is linear in x, so the int8 quantization error there
    is cancelled exactly by a host-computed per-sample correction, folded
    together with all constant terms (value bias, missing-table means,
    predictor bias) into one [L, col] array added at output time.

Math is the same folding as v1: scores y[key,(h,q)] = x_key . U[(h,q)] with
U = scale * Wk^T q(missing_table); per-key per-head attention mass a9 feeds
block one-hot expansion ab; xw[d,(h,col)] = sum_key x[key,d] ab[key,(h,col)]
contracts against (W2_h Wv_h)/S and pred/S to yield logits directly. A 9th
"head" carries the masked doc-mean. Output columns are block-packed; the
host scatters them back to sample order.
"""

import math
import os
import tempfile
from contextlib import ExitStack

import ml_dtypes
import numpy as np

# Re-jitting the dispatch wrapper every call costs ~0.5s in XLA/neuronx
# recompilation; the persistent cache turns that into a fast disk hit.
try:
    import jax
    _cdir = os.path.join(tempfile.gettempdir(), "jax_comp_cache_kernel")
    os.makedirs(_cdir, exist_ok=True)
    jax.config.update("jax_compilation_cache_dir", _cdir)
    jax.config.update("jax_persistent_cache_min_compile_time_secs", 0.0)
    jax.config.update("jax_persistent_cache_min_entry_size_bytes", 0)
except Exception:
    pass

import concourse.bass as bass
import concourse.bacc as bacc_mod
import concourse.mybir as mybir
import concourse.tile as tile
from concourse.bass_utils import run_bass_kernel_spmd

D, H, HD, S, L, B = 768, 8, 96, 23, 50, 4096
NCORES = 8
BC = B // NCORES              # samples per core
NBLK = 24                     # packed blocks per core
NKEY = 256                    # key-row slots per block (2 tiles of 128)
MS = 26                       # max samples per block
NSC = NBLK * MS               # output column slots per core
NH = 9                        # 8 attention heads + 1 doc-mean "head"
NEG = -30000.0
USCALE = 256.0
DELTA = 4.5 / 127.0           # int8 quantization step for x

F32 = mybir.dt.float32
BF16 = mybir.dt.bfloat16
FP8 = mybir.dt.float8e4
I8 = mybir.dt.int8
BF = ml_dtypes.bfloat16
F8 = ml_dtypes.float8_e4m3


def _pack_core(cm):
    """Best-fit-decreasing bin packing of 512 samples into NBLK blocks.

    cm: [BC, S] float 0/1 existing mask. Returns (bins, cnt) where bins is a
    list of NBLK lists of sample indices (placement order = local column).
    """
    cnt = cm.sum(1).astype(np.int64)
    order = np.argsort(-cnt, kind="stable")
    keys_used = np.zeros(NBLK, np.int64)
    samp_used = np.zeros(NBLK, np.int64)
    bins = [[] for _ in range(NBLK)]
    for s in order:
        ok = (keys_used + cnt[s] <= NKEY) & (samp_used < MS)
        if not ok.any():
            raise RuntimeError("bin packing infeasible; raise NBLK")
        cand = np.where(ok)[0]
        j = cand[np.argmax(keys_used[cand])]
        bins[j].append(int(s))
        keys_used[j] += cnt[s]
        samp_used[j] += 1
    return bins, cnt


def _host_prep(cls_emb, missing_table, in_proj_w, in_proj_b,
               out_proj_w, out_proj_b, pred_w, pred_b, exist_mask):
    f32 = np.float32
    x = np.asarray(cls_emb, f32)
    mt = np.asarray(missing_table, f32)
    ipw = np.asarray(in_proj_w, f32)
    ipb = np.asarray(in_proj_b, f32)
    opw = np.asarray(out_proj_w, f32)
    opb = np.asarray(out_proj_b, f32)
    pw = np.asarray(pred_w, f32)
    pb = np.asarray(pred_b, f32)
    em = np.asarray(exist_mask)

    Wq, Wk, Wv = ipw[0:D], ipw[D:2 * D], ipw[2 * D:3 * D]
    bq, bk, bv = ipb[0:D], ipb[D:2 * D], ipb[2 * D:3 * D]
    scale = 1.0 / math.sqrt(HD)
    qm = mt @ Wq.T + bq
    qh = qm.reshape(S, H, HD)
    Wk3 = Wk.reshape(H, HD, D)
    Wv3 = Wv.reshape(H, HD, D)
    U = (np.einsum('hij,qhi->hqj', Wk3, qh) * scale).reshape(H * S, D)
    c0 = (np.einsum('qhi,hi->hq', qh, bk.reshape(H, HD)) * scale).reshape(H * S)
    W2 = pw @ opw
    Mcat = np.einsum('lhi,hid->hld', W2.reshape(L, H, HD), Wv3).reshape(H * L, D) / S
    vbs = ((W2 @ bv + pw @ opb) / S).astype(f32)          # [L] per updated query
    wpts = ((pw @ mt.sum(0)) / S).astype(f32)             # [L] full-table mean

    UT = np.ascontiguousarray(U.T * USCALE).astype(F8)    # [768, 184]
    MT = np.ascontiguousarray(
        np.concatenate([Mcat.T, (pw / S).T], axis=1)).astype(BF)  # [768, 450]
    cn2 = np.stack([c0 * USCALE,
                    np.full(H * S, NEG * USCALE, f32)]).astype(BF)  # [2, 184]
    idbf = np.eye(128, dtype=BF)
    iota_mat = np.ascontiguousarray(
        np.broadcast_to(np.arange(MS, dtype=f32), (128, MS)))

    m = em.astype(f32)                                    # [B, S]
    hasany = (m.sum(1) > 0).astype(f32)
    u = (1.0 - m) * hasany[:, None]
    nupd = u.sum(1)

    # int8 quantized x and the exact doc-mean correction (linear in x)
    xq = np.clip(np.rint(x * (1.0 / DELTA)), -127, 127).astype(np.int8)
    errsum = (np.einsum('bsd,bs->bd', x, m, optimize=True)
              - DELTA * np.einsum('bsd,bs->bd', xq.astype(f32), m, optimize=True))
    corr_all = (errsum @ pw.T) / S \
        + nupd[:, None] * vbs + (1.0 - hasany)[:, None] * wpts + pb  # [B, L]

    shards = []
    colmaps = []
    for c in range(NCORES):
        b0 = c * BC
        cm = m[b0:b0 + BC]
        bins, cnt = _pack_core(cm)
        xpack = np.zeros((NBLK * NKEY, D), np.int8)
        sampcol = np.full((128, NBLK * 2), -1.0, f32)
        realcol = np.zeros((128, NBLK * 2), f32)
        rc2 = np.zeros((2, NBLK * NKEY), f32)
        urpk = np.zeros((MS, NBLK * S), f32)
        corrpk = np.zeros((L, NSC), f32)
        colmap = np.zeros(BC, np.int64)
        for bI, samples in enumerate(bins):
            r = 0
            for j, s in enumerate(samples):
                g = b0 + s
                secs = np.nonzero(cm[s])[0]
                n = len(secs)
                rr = bI * NKEY + r
                xpack[rr:rr + n] = xq[g, secs]
                pidx = np.arange(r, r + n)
                sampcol[pidx % 128, bI * 2 + pidx // 128] = j
                realcol[pidx % 128, bI * 2 + pidx // 128] = 1.0
                rc2[0, rr:rr + n] = 1.0
                urpk[j, bI * S:(bI + 1) * S] = u[g]
                corrpk[:, bI * MS + j] = corr_all[g]
                colmap[s] = bI * MS + j
                r += n
        rc2[1] = 1.0 - rc2[0]
        xqh = np.ascontiguousarray(
            xpack.reshape(NBLK, 2, 128, D).transpose(2, 0, 1, 3)
        ).reshape(128, NBLK * 2 * D)
        pieces = {
            "xq": xqh, "ut": UT, "mt": MT,
            "corr": corrpk.astype(BF), "urpk": urpk.astype(BF),
            "rc2": rc2.astype(BF), "cn2": cn2,
            "sampcol": sampcol.astype(BF), "realcol": realcol.astype(BF),
            "iota": iota_mat.astype(BF), "idbf": idbf,
        }
        buf = np.empty(_AUX_BYTES, np.uint8)
        for name, _, _, ob, nb in _AUX_LAYOUT:
            buf[ob:ob + nb] = pieces[name].reshape(-1).view(np.uint8)
        shards.append({
            "aux": buf.view(BF).reshape(1, -1),
            # extra key, ignored by run_bass_kernel_spmd (only declared
            # input names are read); used by _run to unscatter columns
            "colmap": colmap,
        })
    return shards


def _aux_layout():
    """(name, np_dtype_bytes, shape, byte_offset, byte_count) for each piece
    packed into the single bf16 "aux" input tensor."""
    items = [
        ("xq", 1, (128, NBLK * 2 * D)),
        ("ut", 1, (D, H * S)),
        ("mt", 2, (D, NH * L)),
        ("corr", 2, (L, NSC)),
        ("urpk", 2, (MS, NBLK * S)),
        ("rc2", 2, (2, NBLK * NKEY)),
        ("cn2", 2, (2, H * S)),
        ("sampcol", 2, (128, NBLK * 2)),
        ("realcol", 2, (128, NBLK * 2)),
        ("iota", 2, (128, MS)),
        ("idbf", 2, (128, 128)),
    ]
    out = []
    off = 0
    for name, isz, shape in items:
        nb = isz * int(np.prod(shape))
        assert nb % 2 == 0
        out.append((name, isz, shape, off, nb))
        off += nb
    return out, off


_AUX_LAYOUT, _AUX_BYTES = _aux_layout()


def _build_program():
    nc = bacc_mod.Bacc("TRN2", target_bir_lowering=False, debug=False)
    aux_d = nc.dram_tensor("aux", [1, _AUX_BYTES // 2], BF16,
                           kind="ExternalInput").ap()

    def aux_view(name):
        (_, isz, shape, ob, nb) = next(e for e in _AUX_LAYOUT if e[0] == name)
        v = aux_d[:, ob // 2:(ob + nb) // 2]
        if isz == 1:
            v = v.bitcast(I8 if name == "xq" else FP8)
        return v.rearrange("o (p f) -> (o p) f", p=shape[0])

    xq_d = aux_view("xq")
    sampcol_d = aux_view("sampcol")
    realcol_d = aux_view("realcol")
    rc2_d = aux_view("rc2")
    cn2_d = aux_view("cn2")
    urpk_d = aux_view("urpk")
    corr_d = aux_view("corr")
    iota_d = aux_view("iota")
    idbf_d = aux_view("idbf")
    ut_d = aux_view("ut")
    mt_d = aux_view("mt")
    out_d = nc.dram_tensor("logitsT", [L, NSC], F32, kind="ExternalOutput").ap()

    EXP = mybir.ActivationFunctionType.Exp
    COPY = mybir.ActivationFunctionType.Copy
    HS = H * S

    def mm(out, lhsT, rhs, start, stop):
        nc.tensor.matmul(out, lhsT, rhs, start=start, stop=stop)

    with tile.TileContext(nc) as tc, ExitStack() as ctx:
        cpool = ctx.enter_context(tc.tile_pool(name="consts", bufs=1))
        xip = ctx.enter_context(tc.tile_pool(name="xi", bufs=3))
        xnp = ctx.enter_context(tc.tile_pool(name="xn", bufs=2))
        xtp = ctx.enter_context(tc.tile_pool(name="xt", bufs=2))
        ohp = ctx.enter_context(tc.tile_pool(name="oh", bufs=2))
        ohtp = ctx.enter_context(tc.tile_pool(name="oht", bufs=2))
        ewp = ctx.enter_context(tc.tile_pool(name="ew", bufs=2))
        wvp = ctx.enter_context(tc.tile_pool(name="wv", bufs=2))
        smp = ctx.enter_context(tc.tile_pool(name="small", bufs=4))
        abp = ctx.enter_context(tc.tile_pool(name="ab", bufs=2))
        xsp = ctx.enter_context(tc.tile_pool(name="xs", bufs=2))
        outp = ctx.enter_context(tc.tile_pool(name="out", bufs=1))
        tpp = ctx.enter_context(tc.tile_pool(name="tpps", bufs=2, space="PSUM"))
        top = ctx.enter_context(tc.tile_pool(name="topsum", bufs=1, space="PSUM"))
        yp = ctx.enter_context(tc.tile_pool(name="ypsum", bufs=2, space="PSUM"))
        spp = ctx.enter_context(tc.tile_pool(name="spsum", bufs=2, space="PSUM"))
        xwp = ctx.enter_context(tc.tile_pool(name="xwpsum", bufs=1, space="PSUM"))

        def cload(name, shape, src, cdt=F32):
            t = cpool.tile(shape, cdt, tag=name, name=name)
            nc.sync.dma_start(t[:], src)
            return t

        ut_sb = [cload(f"ut{dc}", [128, HS], ut_d[dc * 128:(dc + 1) * 128, :], FP8)
                 for dc in range(6)]
        mt_sb = [cload(f"mt{dc}", [128, NH * L], mt_d[dc * 128:(dc + 1) * 128, :],
                       BF16) for dc in range(6)]
        rc2_sb = cload("rc2", [2, NBLK * NKEY], rc2_d[:, :], BF16)
        cn2_sb = cload("cn2", [2, HS], cn2_d[:, :], BF16)
        urpk_sb = cload("urpk", [MS, NBLK * S], urpk_d[:, :], BF16)
        corr_sb = cload("corr", [L, NSC], corr_d[:, :], BF16)
        iota_sb = cload("iota", [128, MS], iota_d[:, :], BF16)
        idbf_sb = cload("idbf", [128, 128], idbf_d[:, :], BF16)
        sampcol_sb = cload("sampcol", [128, NBLK * 2], sampcol_d[:, :], BF16)
        realcol_sb = cload("realcol", [128, NBLK * 2], realcol_d[:, :], BF16)
        outT = outp.tile([L, NSC], F32, tag="outT", name="outT")

        tiles = {}

        def emit_load(b):
            xi = xip.tile([128, 2, D], I8, tag="xi", name=f"xi{b}")
            nc.sync.dma_start(xi[:], xq_d[:, b * 2 * D:(b + 1) * 2 * D]
                              .rearrange("p (t d) -> p t d", t=2))
            tiles.setdefault(b, {})["xi"] = xi

        def emit_cast(b):
            xi = tiles[b]["xi"]
            xnb = xnp.tile([128, 2, D], BF16, tag="xnb", name=f"xnb{b}")
            nc.scalar.activation(xnb[:], xi[:], COPY, scale=DELTA)
            tiles[b]["xnb"] = xnb

        def emit_xts(b):
            xnb = tiles[b]["xnb"]
            xts = xtp.tile([128, 6, NKEY], FP8, tag="xts", name=f"xts{b}")
            for t in range(2):
                for dc in range(6):
                    tp = tpp.tile([128, 128], BF16, tag="tp", name=f"tp{b}_{t}_{dc}")
                    nc.tensor.transpose(tp[:], xnb[:, t, dc * 128:(dc + 1) * 128],
                                        idbf_sb[:])
                    if dc % 2 == 0:
                        nc.scalar.copy(xts[:, dc, t * 128:(t + 1) * 128], tp[:])
                    else:
                        nc.vector.tensor_copy(xts[:, dc, t * 128:(t + 1) * 128],
                                              tp[:])
            tiles[b]["xts"] = xts

        def emit_oh(b):
            oh = ohp.tile([128, 2, MS], BF16, tag="oh", name=f"oh{b}")
            ohT = ohtp.tile([MS, 2, 128], BF16, tag="ohT", name=f"ohT{b}")
            for t in range(2):
                nc.vector.tensor_tensor(
                    oh[:, t, :], iota_sb[:],
                    sampcol_sb[:, b * 2 + t:b * 2 + t + 1].broadcast_to([128, MS]),
                    op=mybir.AluOpType.is_equal)
                tpo = top.tile([MS, 128], BF16, tag="tpo", name=f"tpo{b}_{t}")
                nc.tensor.transpose(tpo[:], oh[:, t, :], idbf_sb[:])
                nc.vector.tensor_copy(ohT[:, t, :], tpo[:])
            tiles[b]["oh"] = oh
            tiles[b]["ohT"] = ohT

        def emit_scores(b):
            xts = tiles[b]["xts"]
            ewT = ewp.tile([128, 2, HS], BF16, tag="ewT", name=f"ewT{b}")
            for t in range(2):
                y = yp.tile([128, HS], F32, tag="y", name=f"y{b}_{t}")
                with tc.high_priority():
                    for dc in range(6):
                        mm(y[:], xts[:, dc, t * 128:(t + 1) * 128], ut_sb[dc][:],
                           start=(dc == 0), stop=False)
                    r0 = b * NKEY + t * 128
                    mm(y[:], rc2_sb[:, r0:r0 + 128], cn2_sb[:],
                       start=False, stop=True)
                nc.scalar.activation(ewT[:, t, :], y[:], EXP, scale=1.0 / USCALE)
            tiles[b]["ewT"] = ewT

        def emit_den(b):
            oh, ewT = tiles[b]["oh"], tiles[b]["ewT"]
            dpsf = spp.tile([128, HS], F32, tag="sp", name=f"dps{b}")
            dps = dpsf[0:MS, :]
            mm(dps, oh[:, 0, :], ewT[:, 0, :], start=True, stop=False)
            mm(dps, oh[:, 1, :], ewT[:, 1, :], start=False, stop=True)
            up = smp.tile([MS, HS], F32, tag="up", name=f"up{b}")
            nc.vector.tensor_scalar_add(up[:], dps, 1e-30)
            nc.vector.reciprocal(up[:], up[:])
            upu = smp.tile([MS, HS], BF16, tag="upu", name=f"upu{b}")
            nc.gpsimd.tensor_mul(
                upu[:].rearrange("p (h k) -> p h k", k=S),
                up[:].rearrange("p (h k) -> p h k", k=S),
                urpk_sb[:, b * S:(b + 1) * S]
                .rearrange("p (o k) -> p o k", o=1).broadcast_to([MS, H, S]))
            tiles[b]["upu"] = upu

        def emit_w(b):
            ohT, ewT, upu = tiles[b]["ohT"], tiles[b]["ewT"], tiles[b]["upu"]
            wv = wvp.tile([128, 2, HS], BF16, tag="wv", name=f"wv{b}")
            for t in range(2):
                ue = spp.tile([128, HS], F32, tag="sp", name=f"ue{b}_{t}")
                mm(ue[:], ohT[:, t, :], upu[:], start=True, stop=True)
                nc.vector.tensor_mul(wv[:, t, :], ewT[:, t, :], ue[:])
            tiles[b]["wv"] = wv

        def emit_a9ab(b):
            wv, oh = tiles[b]["wv"], tiles[b]["oh"]
            a9 = smp.tile([128, 2, NH], F32, tag="a9", name=f"a9_{b}")
            nc.vector.tensor_reduce(
                a9[:, :, 0:H].rearrange("p t (h o) -> p t h o", o=1),
                wv[:].rearrange("p t (h k) -> p t h k", k=S),
                axis=mybir.AxisListType.X, op=mybir.AluOpType.add)
            nc.gpsimd.tensor_copy(
                a9[:, :, H:NH],
                realcol_sb[:, b * 2:b * 2 + 2].rearrange("p (t o) -> p t o", o=1))
            ab = abp.tile([128, 2, NH, MS], BF16, tag="ab", name=f"ab{b}")
            nc.gpsimd.tensor_copy(
                ab[:],
                oh[:].rearrange("p t (o m) -> p t o m", o=1)
                .broadcast_to([128, 2, NH, MS]))
            nc.gpsimd.tensor_mul(
                ab[:], ab[:],
                a9[:].rearrange("p t (h o) -> p t h o", o=1)
                .broadcast_to([128, 2, NH, MS]))
            tiles[b]["ab"] = ab

        def emit_xw(b):
            xnb, ab = tiles[b]["xnb"], tiles[b]["ab"]
            xs = xsp.tile([128, 6, NH * MS], BF16, tag="xs", name=f"xs{b}")
            for dc in range(6):
                xw = xwp.tile([128, NH * MS], F32, tag="xw", name=f"xw{b}_{dc}")
                mm(xw[:], xnb[:, 0, dc * 128:(dc + 1) * 128], ab[:, 0],
                   start=True, stop=False)
                mm(xw[:], xnb[:, 1, dc * 128:(dc + 1) * 128], ab[:, 1],
                   start=False, stop=True)
                if dc % 2 == 0:
                    nc.scalar.copy(xs[:, dc, :], xw[:])
                else:
                    nc.vector.tensor_copy(xs[:, dc, :], xw[:])
            tiles[b]["xs"] = xs

        def emit_mh(b):
            xs = tiles[b]["xs"]
            ctf = spp.tile([128, HS], F32, tag="sp", name=f"ct{b}")
            ct = ctf[0:L, 0:MS]
            first = True
            for dc in range(6):
                for h in range(NH):
                    mm(ct, mt_sb[dc][:, h * L:(h + 1) * L],
                       xs[:, dc, h * MS:(h + 1) * MS],
                       start=first, stop=(dc == 5 and h == NH - 1))
                    first = False
            nc.vector.tensor_tensor(
                outT[:, b * MS:(b + 1) * MS], ct,
                corr_sb[:, b * MS:(b + 1) * MS], op=mybir.AluOpType.add)
            del tiles[b]

        for b in range(min(2, NBLK)):
            emit_load(b)
        emit_cast(0)
        for b in range(NBLK):
            if b + 2 < NBLK:
                emit_load(b + 2)
            if b + 1 < NBLK:
                emit_cast(b + 1)
            emit_xts(b)
            emit_oh(b)
            emit_scores(b)
            emit_den(b)
            emit_w(b)
            emit_a9ab(b)
            emit_xw(b)
            emit_mh(b)

        nc.sync.dma_start(out_d[:, :], outT[:])
    nc.compile()
    return nc


_CACHED = {}


def _get_program():
    if "nc" not in _CACHED:
        _CACHED["nc"] = _build_program()
        _install_memo_dispatch()
    return _CACHED["nc"]


def _install_memo_dispatch():
    """Memoize the jitted dispatch closure for our program.

    bass2jax.run_bass_via_pjrt rebuilds its jax.jit wrapper on every call,
    which re-pays trace + compilation-cache lookup each dispatch. For our
    (single, immutable) program we build the wrapper once and reuse it;
    any other program falls through to the original implementation.
    """
    import jax
    from jax.sharding import Mesh, PartitionSpec
    from jax.experimental.shard_map import shard_map
    from concourse import bass2jax as b2j

    if _CACHED.get("patched"):
        return
    orig = b2j.run_bass_via_pjrt

    def build(nc, n_cores):
        b2j.install_neuronx_cc_hook()
        partition_name = (nc.partition_id_tensor.name
                          if nc.partition_id_tensor else None)
        in_names, out_names, out_avals, zero_shapes = [], [], [], []
        for alloc in nc.m.functions[0].allocations:
            if not isinstance(alloc, mybir.MemoryLocationSet):
                continue
            name = alloc.memorylocations[0].name
            if alloc.kind == "ExternalInput":
                if name != partition_name:
                    in_names.append(name)
            elif alloc.kind == "ExternalOutput":
                shape = tuple(alloc.tensor_shape)
                dtype = mybir.dt.np(alloc.dtype)
                out_names.append(name)
                out_avals.append(jax.core.ShapedArray(shape, dtype))
                zero_shapes.append((shape, dtype))
        n_params = len(in_names)
        all_names = list(in_names) + list(out_names)
        if partition_name is not None:
            all_names.append(partition_name)
        donate = tuple(range(n_params, n_params + len(out_avals)))

        def _body(*args):
            operands = list(args)
            if partition_name is not None:
                operands.append(b2j.partition_id_tensor())
            outs = b2j._bass_exec_p.bind(
                *operands, out_avals=tuple(out_avals),
                in_names=tuple(all_names), out_names=tuple(out_names),
                lowering_input_output_aliases=(),
                sim_require_finite=True, sim_require_nnan=True, nc=nc)
            return tuple(outs)

        devices = jax.devices()[:n_cores]
        mesh = Mesh(np.asarray(devices), ("core",))
        nspecs = n_params + len(out_avals)
        sharded = jax.jit(
            shard_map(_body, mesh=mesh,
                      in_specs=(PartitionSpec("core"),) * nspecs,
                      out_specs=(PartitionSpec("core"),) * len(out_names),
                      check_rep=False),
            donate_argnums=donate, keep_unused=True)
        return sharded, in_names, out_names, out_avals, zero_shapes

    def patched(nc, in_maps, n_cores):
        if nc is not _CACHED.get("nc") or nc.dbg_addr is not None                 or n_cores != NCORES:
            return orig(nc, in_maps, n_cores)
        if "disp" not in _CACHED:
            _CACHED["disp"] = build(nc, n_cores)
        sharded, in_names, out_names, out_avals, zero_shapes = _CACHED["disp"]
        concat_in = [
            np.concatenate([np.asarray(m[name]) for m in in_maps], axis=0)
            for name in in_names]
        concat_zeros = [
            np.zeros((n_cores * s[0], *s[1:]), dt) for s, dt in zero_shapes]
        out_arrs = sharded(*concat_in, *concat_zeros)
        return [
            {name: np.asarray(out_arrs[i]).reshape(
                n_cores, *out_avals[i].shape)[c]
             for i, name in enumerate(out_names)}
            for c in range(n_cores)]

    b2j.run_bass_via_pjrt = patched
    _CACHED["patched"] = True


def _run(inputs, trace=False):
    shards = _host_prep(**inputs)
    nc = _get_program()
    res = run_bass_kernel_spmd(nc, shards, list(range(NCORES)), trace=trace)
    full = np.empty((B, L), np.float32)
    for c in range(NCORES):
        oT = res.results[c]["logitsT"]          # [L, NSC]
        full[c * BC:(c + 1) * BC] = oT[:, shards[c]["colmap"]].T
    return full, res


def kernel(**inputs):
    out, _ = _run(inputs, trace=False)
    return out


def run_traced(inputs):
    return _run(inputs, trace=True)
